# revision 1
# baseline (speedup 1.0000x reference)
"""Trainium2 Bass kernel for GroupedQuerySelfAttention.

Problem: B=2, N=2048, D=2048, H=8 kv-heads, G=4 (32 query heads), C=64.
  q = (x @ Wq) / sqrt(32);  kv = x @ Wkv;  k, v = split(kv)
  per (b, h, g): S = Qg K^T;  A = softmax(S);  O = A V
  out = concat_heads(O) @ Wp + bp

Sharding: 8 cores = 2 batches x 4 query-chunks of 512 rows. Each core
computes K/V for its whole batch (duplicated within the 4-core group --
no collectives), attention for its 512 query rows over all 32 heads,
and its 512 rows of the output projection. Host concatenates.

Layouts (per core):
  xT   [d, n]  : x transposed, built via PE transpose (fp32)
  Q^T  [j, n]  : lhsT = Wq[d-block, j-block], rhs = xT[d-block, nq]
  K^T  [j, n]  : lhsT = Wkv[d-block, j-block], rhs = xT[d-block, n]
  V~   [n, 65] : per head: V columns + a ones column (gives softmax
                 denominators for free in the PV matmul, psum row 64)
  S^T  [s, q]  : lhsT = K^T[c, s-block], rhs = Q^T[c, q]  (contraction c=64)
  E^T  = exp(S^T / sqrt(32))  on ACT, scale folded into the activation
  O'^T [65, q] : lhsT = V~[s-block, 65], rhs = E^T[s-block, q], accum over s
  o^T  [j, q]  : O'^T rows 0:64 * recip(row 64), broadcast via tiny DMA
  out  [q, d]  : lhsT = o^T[j-block, q-block], rhs = Wp[j-block, d-chunk],
                 bias added during psum evacuation (DMA-broadcast bp)
"""

import numpy as np
from contextlib import ExitStack

import concourse.bass as bass
import concourse.tile as tile
from concourse import bacc, mybir
from concourse.bass_utils import run_bass_kernel_spmd
from concourse.masks import make_identity

P = 128
B, N, D = 2, 2048, 2048
H, G, C = 8, 4, 64
NQ = 512                      # query rows per core
DB = D // P                   # 16 d-blocks
NB = N // P                   # 16 seq blocks
QB = NQ // P                  # 4 query blocks
SCALE = float(1.0 / np.sqrt(H * G))
F32 = mybir.dt.float32
F32R = mybir.dt.float32r
AF = mybir.ActivationFunctionType
BF16 = mybir.dt.bfloat16

USE_F32R = True


def _r(ap):
    if ap.dtype == F32 and USE_F32R:
        return ap.bitcast(F32R)
    return ap


def build_program(n_cores=8, phases="ABCD"):
    nc = bacc.Bacc("TRN2", target_bir_lowering=False, debug=False,
                   num_devices=n_cores)
    xb = nc.dram_tensor("xb", [N, D], F32, kind="ExternalInput").ap()
    xq = nc.dram_tensor("xq", [NQ, D], F32, kind="ExternalInput").ap()
    wq = nc.dram_tensor("wq", [D, D], BF16, kind="ExternalInput").ap()
    wkv = nc.dram_tensor("wkv", [D, 2 * H * C], BF16, kind="ExternalInput").ap()
    wp = nc.dram_tensor("wp", [D, D], BF16, kind="ExternalInput").ap()
    bp = nc.dram_tensor("bp", [D], F32, kind="ExternalInput").ap()
    out = nc.dram_tensor("out", [NQ, D], F32, kind="ExternalOutput").ap()

    with tile.TileContext(nc) as tc, ExitStack() as top:
        # ---- persistent stores ----
        store = top.enter_context(tc.tile_pool(name="store", bufs=1))
        QT = store.tile([P, DB, NQ], F32R, tag="QT")       # [j, n] 32KB/part
        KT = store.tile([P, H * C // P, N], F32R, tag="KT")  # [j, n] 32KB/part
        Vst = store.tile([P, NB, H, C + 1], F32R, tag="Vst")  # [n, h, 65]
        OT = store.tile([P, DB, NQ], BF16, tag="OT")       # o^T [j, q]
        ident = store.tile([P, P], F32, tag="ident")
        make_identity(nc, ident[:])
        ones = store.tile([P, 1], F32, tag="ones")
        nc.gpsimd.memset(ones[:], 1.0)
        nc.vector.tensor_copy(                            # ones column (f32r)
            Vst[:, :, :, C:C + 1],
            ones[:, None, None, :].to_broadcast((P, NB, H, 1)))
        bpb = store.tile([P, D], F32, tag="bpb")
        nc.sync.dma_start(bpb[:], bp[None, :].to_broadcast((P, D)))

        # ---- phase A: transpose xq, project Q^T ----
        with ExitStack() as ctx:
          if 'A' in phases:
              xqT_p = ctx.enter_context(tc.tile_pool(name="xqT", bufs=1))
              xqT = xqT_p.tile([P, DB, NQ], BF16, tag="xqT")
              with ExitStack() as tctx:
                  xrow = tctx.enter_context(tc.tile_pool(name="xrow", bufs=2))
                  tpsum = tctx.enter_context(
                      tc.tile_pool(name="tpsum", bufs=2, space="PSUM"))
                  for qb in range(QB):
                      xt = xrow.tile([P, D], F32, tag="xrow")
                      nc.sync.dma_start(xt[:], xq[qb * P:(qb + 1) * P, :])
                      for db4 in range(DB // 4):
                          tp = tpsum.tile([P, 4, P], F32, tag="tp")
                          for i in range(4):
                              nc.tensor.transpose(
                                  tp[:, i, :],
                                  xt[:, (db4 * 4 + i) * P:(db4 * 4 + i + 1) * P],
                                  ident[:])
                          nc.vector.tensor_copy(
                              xqT[:, db4 * 4:db4 * 4 + 4,
                                  qb * P:(qb + 1) * P], tp[:])
              wq_p = ctx.enter_context(tc.tile_pool(name="wq", bufs=2))
              qpsum = ctx.enter_context(
                  tc.tile_pool(name="qpsum", bufs=8, space="PSUM"))
              # Q^T stored g-major: block bq = g*4 + h//2, row off (h%2)*64 + c.
              # This aligns Q^T's partition offset with K^T's for every (h, g).
              # The host pre-permutes Wq columns to the same g-major order, so
              # stationary slices stay contiguous.
              for half in range(2):          # wq streamed twice, 8 psums/group
                  psums = [qpsum.tile([P, NQ], F32, tag="qpsum",
                                      name=f"qps{half}_{i}") for i in range(8)]
                  for db in range(DB):
                      wt = wq_p.tile([P, D], BF16, tag="wq")
                      eng = nc.sync if db % 2 == 0 else nc.scalar
                      eng.dma_start(wt[:], wq[db * P:(db + 1) * P, :])
                      for i in range(8):
                          bq = half * 8 + i
                          nc.tensor.matmul(
                              psums[i][:], _r(wt[:, bq * P:(bq + 1) * P]),
                              _r(xqT[:, db, :]),
                              start=(db == 0), stop=(db == DB - 1))
                  for i in range(8):
                      nc.vector.tensor_copy(QT[:, half * 8 + i, :], psums[i][:])

        # ---- phase B: transpose xb chunk-wise, project K^T and V~ ----
        with ExitStack() as ctx:
          if 'B' in phases:
              xrow = ctx.enter_context(tc.tile_pool(name="xrowb", bufs=2))
              tpsum = ctx.enter_context(
                  tc.tile_pool(name="tpsumb", bufs=2, space="PSUM"))
              xbT_p = ctx.enter_context(tc.tile_pool(name="xbT", bufs=2))
              wkv_p = ctx.enter_context(tc.tile_pool(name="wkv", bufs=3))
              kvpsum = ctx.enter_context(
                  tc.tile_pool(name="kvpsum", bufs=6, space="PSUM"))

              for ch in range(N // NQ):          # 4 chunks of 512 seq rows
                  xbT = xbT_p.tile([P, DB, NQ], BF16, tag="xbT")
                  for qb in range(QB):
                      xt = xrow.tile([P, D], F32, tag="xrowb")
                      nc.scalar.dma_start(
                          xt[:], xb[ch * NQ + qb * P:ch * NQ + (qb + 1) * P, :])
                      for db4 in range(DB // 4):
                          tp = tpsum.tile([P, 4, P], F32, tag="tpb")
                          for i in range(4):
                              nc.tensor.transpose(
                                  tp[:, i, :],
                                  xt[:, (db4 * 4 + i) * P:(db4 * 4 + i + 1) * P],
                                  ident[:])
                          nc.vector.tensor_copy(
                              xbT[:, db4 * 4:db4 * 4 + 4,
                                  qb * P:(qb + 1) * P], tp[:])
                  # K^T: 4 j-blocks x 512 n, accumulate over d
                  kps = [kvpsum.tile([P, NQ], F32, tag="kvp", name=f"kps{ch}_{i}") for i in range(4)]
                  for db in range(DB):
                      wt = wkv_p.tile([P, H * C], BF16, tag="wkvk")
                      eng = nc.sync if db % 2 == 0 else nc.scalar
                      eng.dma_start(wt[:], wkv[db * P:(db + 1) * P, :H * C])
                      for jb in range(4):
                          nc.tensor.matmul(
                              kps[jb][:], _r(wt[:, jb * P:(jb + 1) * P]),
                              _r(xbT[:, db, :]),
                              start=(db == 0), stop=(db == DB - 1))
                  for jb in range(4):
                      nc.vector.tensor_copy(KT[:, jb, ch * NQ:(ch + 1) * NQ],
                                            kps[jb][:])
                  # V: 4 n-blocks x 512 j, accumulate over d
                  vps = [kvpsum.tile([P, NQ], F32, tag="kvp", name=f"vps{ch}_{i}") for i in range(4)]
                  for db in range(DB):
                      wt = wkv_p.tile([P, H * C], BF16, tag="wkvv")
                      eng = nc.sync if db % 2 == 0 else nc.scalar
                      eng.dma_start(wt[:], wkv[db * P:(db + 1) * P, H * C:])
                      for nb4 in range(4):
                          nc.tensor.matmul(
                              vps[nb4][:], _r(xbT[:, db, nb4 * P:(nb4 + 1) * P]),
                              _r(wt[:]),
                              start=(db == 0), stop=(db == DB - 1))
                  for nb4 in range(4):
                      sb = ch * 4 + nb4
                      for h in range(H):
                          nc.vector.tensor_copy(
                              Vst[:, sb, h, :C],
                              vps[nb4][:, h * C:(h + 1) * C])

        # ---- phase C: attention per (h, g) ----
        with ExitStack() as ctx:
          if 'C' in phases:
              qkpsum = ctx.enter_context(
                  tc.tile_pool(name="qkpsum", bufs=5, space="PSUM"))
              pvpsum = ctx.enter_context(
                  tc.tile_pool(name="pvpsum", bufs=2, space="PSUM"))
              e_p = ctx.enter_context(tc.tile_pool(name="epool", bufs=24))
              rec_p = ctx.enter_context(tc.tile_pool(name="rec", bufs=3))
              rb_p = ctx.enter_context(tc.tile_pool(name="rb", bufs=3))
              dram_p = ctx.enter_context(
                  tc.tile_pool(name="dramrec", bufs=4, space="DRAM"))

              ot_p = ctx.enter_context(tc.tile_pool(name="otmp", bufs=3))
              for h in range(H):
                  for g in range(G):
                      off = (h % 2) * C               # K^T and Q^T row offset
                      kt_jb = h // 2
                      qt_jb = g * 4 + h // 2          # g-major Q^T block
                      e_tiles = []
                      for sb in range(NB):
                          qk = qkpsum.tile([P, NQ], F32, tag="qk")
                          nc.tensor.matmul(
                              qk[:],
                              _r(KT[off:off + C, kt_jb, sb * P:(sb + 1) * P]),
                              _r(QT[off:off + C, qt_jb, :]),
                              start=True, stop=True)
                          et = e_p.tile([P, NQ], F32R, tag="E")
                          nc.scalar.activation(et[:], qk[:], AF.Exp, scale=SCALE)
                          e_tiles.append(et)
                      pv = pvpsum.tile([C + 1, NQ], F32, tag="pv")
                      for sb in range(NB):
                          nc.tensor.matmul(
                              pv[:], _r(Vst[:, sb, h, :]), _r(e_tiles[sb][:]),
                              start=(sb == 0), stop=(sb == NB - 1))
                      rec = rec_p.tile([C + 1, NQ], F32, tag="rec")
                      nc.vector.reciprocal(rec[C:C + 1, :], pv[C:C + 1, :])
                      # partition-broadcast rec via a DRAM bounce (DMA cannot
                      # read SBUF with zero partition step, DRAM is fine)
                      recd = dram_p.tile([1, NQ], F32, tag="recd")
                      nc.sync.dma_start(recd[:], rec[C:C + 1, :])
                      rb = rb_p.tile([C, NQ], F32, tag="rb")
                      nc.sync.dma_start(rb[:], recd[:].to_broadcast((C, NQ)))
                      # o^T rows for (h,g) live at j = h*G*C + g*C (+64 for odd
                      # g); DVE can't shift partitions, so odd halves go via a
                      # small SBUF->SBUF DMA.
                      oj = h * G * C + g * C
                      o_jb, o_off = oj // P, oj % P
                      if o_off == 0:
                          nc.vector.tensor_mul(OT[:C, o_jb, :], pv[:C, :], rb[:])
                      else:
                          ot = ot_p.tile([C, NQ], BF16, tag="otmp")
                          nc.vector.tensor_mul(ot[:], pv[:C, :], rb[:])
                          nc.sync.dma_start(OT[o_off:o_off + C, o_jb, :], ot[:])

        # ---- phase D: output projection + bias ----
        with ExitStack() as ctx:
          if 'D' in phases:
              wp_p = ctx.enter_context(tc.tile_pool(name="wp", bufs=2))
              opsum = ctx.enter_context(
                  tc.tile_pool(name="opsum", bufs=3, space="PSUM"))
              osb_p = ctx.enter_context(tc.tile_pool(name="osb", bufs=3))

              for ob in range(4):                 # output col chunks of 512
                  wpt = wp_p.tile([P, DB, NQ], BF16, tag="wp")
                  for jb in range(DB):
                      eng = nc.sync if jb % 2 == 0 else nc.scalar
                      eng.dma_start(
                          wpt[:, jb, :],
                          wp[jb * P:(jb + 1) * P, ob * NQ:(ob + 1) * NQ])
                  for qb in range(QB):
                      ps = opsum.tile([P, NQ], F32, tag="op")
                      for jb in range(DB):
                          nc.tensor.matmul(
                              ps[:], _r(OT[:, jb, qb * P:(qb + 1) * P]),
                              _r(wpt[:, jb, :]),
                              start=(jb == 0), stop=(jb == DB - 1))
                      osb = osb_p.tile([P, NQ], F32, tag="osb")
                      nc.vector.tensor_add(osb[:], ps[:],
                                           bpb[:, ob * NQ:(ob + 1) * NQ])
                      nc.sync.dma_start(
                          out[qb * P:(qb + 1) * P, ob * NQ:(ob + 1) * NQ],
                          osb[:])

    nc.compile()
    return nc


_nc_cache = None


def kernel(x, Wq, Wkv, Wp, bp):
    global _nc_cache
    if _nc_cache is None:
        _nc_cache = build_program()
    nc = _nc_cache
    x = np.ascontiguousarray(np.asarray(x, dtype=np.float32))
    import ml_dtypes
    # permute Wq columns to g-major head order (see build_program phase A)
    Wq = np.ascontiguousarray(
        np.asarray(Wq, dtype=np.float32)
        .reshape(D, H, G, C).transpose(0, 2, 1, 3).reshape(D, D)
        .astype(ml_dtypes.bfloat16))
    Wkv = np.asarray(Wkv, dtype=np.float32).astype(ml_dtypes.bfloat16)
    Wp = np.asarray(Wp, dtype=np.float32).astype(ml_dtypes.bfloat16)
    bp = np.ascontiguousarray(np.asarray(bp, dtype=np.float32))

    in_maps = []
    for c in range(8):
        b, qc = c // 4, c % 4
        in_maps.append({
            "xb": x[b],
            "xq": x[b, qc * NQ:(qc + 1) * NQ],
            "wq": Wq, "wkv": Wkv, "wp": Wp, "bp": bp,
        })
    res = run_bass_kernel_spmd(nc, in_maps, list(range(8)))
    outp = np.empty((B, N, D), np.float32)
    for c in range(8):
        outp[c // 4, (c % 4) * NQ:(c % 4 + 1) * NQ] = res.results[c]["out"]
    return outp



# revision 15
# speedup vs baseline: 1.3374x; 1.3374x over previous
"""Trainium2 Bass kernel for GroupedQuerySelfAttention (v2, restructured).

Problem: B=2, N=2048, D=2048, H=8 kv-heads, G=4 (32 query heads), C=64.
  q = (x @ Wq) / sqrt(32);  kv = x @ Wkv;  k, v = split(kv)
  per (b, h, g): S = Qg K^T;  A = softmax(S);  O = A V
  out = concat_heads(O) @ Wp + bp

Sharding: 8 cores = 2 batches x 4 query-chunks of 512 rows. Each core
computes K/V for its whole batch (duplicated within the 4-core group --
collectives are slower than the duplicated compute under this machine's
cost model), attention for its 512 query rows over all 32 heads, and its
512 rows of the output projection. Host concatenates.

Key layout choices (all matmul inputs bf16; psum f32):
  xT   [d, n]   host-pre-transposed x, so no PE transposes of x
  Q^T  [j, n]   g-major head order (host-permuted Wq) so Q^T and K^T
                partition offsets line up per (h, g)
  K^T  [j, n]
  V~   [s, h, 65]  V columns + ones column (softmax denominators fall
                out of the PV matmul for free)
  S^T  [s, q]   lhsT = K^T slice, rhs = Q^T slice (contraction c=64)
  E^T  = exp(S^T / sqrt(32)) -> bf16, exp'd in [128, 2, 512] groups
  PV:  out O[q, 65] with lhsT = E^T (q-partition output: free dim is
                only 65, halving PE cost vs the [65, q] orientation)
  O accumulated over seq chunks in SBUF f32; per-row 1/denom applied at
  the end; O transposed back to [j, q] via PE for the out projection.

Pipeline: K proj -> Q proj -> V chunk0 -> 4 attention rounds (one per
seq chunk; V chunk ch+1 is projected inside round ch, hidden under the
ACT-engine exp stream which is the phase bottleneck) -> O evac -> out
projection.
"""

import numpy as np
from contextlib import ExitStack

import concourse.bass as bass
import concourse.tile as tile
from concourse import bacc, mybir
from concourse.bass_utils import run_bass_kernel_spmd
from concourse.masks import make_identity

P = 128
B, N, D = 2, 2048, 2048
H, G, C = 8, 4, 64
HG = H * G                     # 32 query heads
NQ = 512                       # query rows per core
DB = D // P                    # 16 d-blocks
NB = N // P                    # 16 seq blocks
QB = NQ // P                   # 4 query blocks
CH = N // NQ                   # 4 seq chunks
SCALE = float(1.0 / np.sqrt(HG))
F32 = mybir.dt.float32
BF16 = mybir.dt.bfloat16
AF = mybir.ActivationFunctionType


def build_program(n_cores=8, dbg=False):
    nc = bacc.Bacc("TRN2", target_bir_lowering=False, debug=False,
                   num_devices=n_cores)
    dbg_t = {}
    if dbg:
        for nm, shp in [("dQT", [P, DB, NQ]), ("dKT", [P, 4, N]),
                        ("dVst", [P, NB, H, C + 1]), ("dOT", [P, DB, NQ])]:
            dbg_t[nm] = nc.dram_tensor(nm, shp, BF16, kind="ExternalOutput").ap()
        dbg_t["dOacc"] = nc.dram_tensor(
            "dOacc", [P, QB, HG // 2, 2, C + 1], F32, kind="ExternalOutput").ap()
    # host-prepared layouts (see kernel() below)
    xt = nc.dram_tensor("xt", [DB, P, N], BF16, kind="ExternalInput").ap()
    xqt = nc.dram_tensor("xqt", [DB, P, NQ], BF16, kind="ExternalInput").ap()
    wq = nc.dram_tensor("wq", [4, DB, P, NQ], BF16, kind="ExternalInput").ap()
    wkv = nc.dram_tensor("wkv", [DB, P, 2, NQ], BF16, kind="ExternalInput").ap()
    wp = nc.dram_tensor("wp", [DB, P, 4, NQ], BF16, kind="ExternalInput").ap()
    bp = nc.dram_tensor("bp", [D], F32, kind="ExternalInput").ap()
    out = nc.dram_tensor("out", [QB, P, 4, NQ], BF16, kind="ExternalOutput").ap()

    with tile.TileContext(nc) as tc, ExitStack() as top:
        per = top.enter_context(tc.tile_pool(name="per", bufs=1))
        identb = per.tile([P, P], BF16, tag="identb")
        make_identity(nc, identb[:])
        ones = per.tile([P, 1], BF16, tag="ones")
        nc.gpsimd.memset(ones[:], 1.0)
        # O accumulator survives from the attention rounds into the tail
        Oacc = top.enter_context(tc.tile_pool(name="Oaccp", bufs=1)).tile(
            [P, QB, H * G // 2, 2, C + 1], F32, tag="Oacc")
        with ExitStack() as main:
            xts = main.enter_context(tc.tile_pool(name="xts", bufs=1))
            xT = xts.tile([P, DB, N], BF16, tag="xT")
            for db in range(DB):
                eng = nc.sync if db % 2 == 0 else nc.gpsimd
                eng.dma_start(xT[:, db, :], xt[db])
            wkvp = main.enter_context(tc.tile_pool(name="wkvp", bufs=1))
            wkv_v = wkvp.tile([P, DB, NQ], BF16, tag="wkv_v")
            for db in range(DB):
                nc.gpsimd.dma_start(wkv_v[:, db, :], wkv[db, :, 1, :])

            QT = main.enter_context(tc.tile_pool(name="QTp", bufs=1)).tile(
                [P, DB, NQ], BF16, tag="QT")
            KT = main.enter_context(tc.tile_pool(name="KTp", bufs=1)).tile(
                [P, H * C // P, N], BF16, tag="KT")
            Vst = main.enter_context(tc.tile_pool(name="Vstp", bufs=1)).tile(
                [P, NB, H, C + 1], BF16, tag="Vst")
            nc.vector.tensor_copy(
                Vst[:, :, :, C:C + 1],
                ones[:, None, None, :].to_broadcast((P, NB, H, 1)))

            # ---- K projection: K^T[j, n] for all 4 chunks ----
            with ExitStack() as s:
                wkp = s.enter_context(tc.tile_pool(name="wkp", bufs=1))
                wkv_k = wkp.tile([P, DB, NQ], BF16, tag="wkv_k")
                for db in range(DB):
                    nc.sync.dma_start(wkv_k[:, db, :], wkv[db, :, 0, :])
                kps = s.enter_context(
                    tc.tile_pool(name="kps", bufs=4, space="PSUM"))
                for ch in range(CH):
                    for jb in range(4):
                        kp = kps.tile([P, NQ], F32, tag="kp")
                        for db in range(DB):
                            nc.tensor.matmul(
                                kp[:], wkv_k[:, db, jb * P:(jb + 1) * P],
                                xT[:, db, ch * NQ:(ch + 1) * NQ],
                                start=(db == 0), stop=(db == DB - 1))
                        nc.vector.tensor_copy(
                            KT[:, jb, ch * NQ:(ch + 1) * NQ], kp[:])

            # ---- Q projection: Q^T[j, n] for this core's 512 rows ----
            with ExitStack() as s:
                xqs = s.enter_context(tc.tile_pool(name="xqs", bufs=1))
                xq = xqs.tile([P, DB, NQ], BF16, tag="xq")
                for db in range(DB):
                    eng = nc.sync if db % 2 == 0 else nc.gpsimd
                    eng.dma_start(xq[:, db, :], xqt[db])
                wqp = s.enter_context(tc.tile_pool(name="wqp", bufs=1))
                qps = s.enter_context(
                    tc.tile_pool(name="qps", bufs=4, space="PSUM"))
                for jc in range(4):
                    wt0 = wqp.tile([P, 8, NQ], BF16, tag="wqh0")
                    wt1 = wqp.tile([P, 8, NQ], BF16, tag="wqh1")
                    for i in range(8):
                        nc.sync.dma_start(wt0[:, i, :], wq[jc, i])
                        nc.gpsimd.dma_start(wt1[:, i, :], wq[jc, 8 + i])
                    for jb in range(4):
                        qp = qps.tile([P, NQ], F32, tag="qp")
                        for db in range(DB):
                            wt = wt0 if db < 8 else wt1
                            nc.tensor.matmul(
                                qp[:], wt[:, db % 8, jb * P:(jb + 1) * P],
                                xq[:, db, :],
                                start=(db == 0), stop=(db == DB - 1))
                        nc.vector.tensor_copy(QT[:, jc * 4 + jb, :], qp[:])

            # ---- V chunk 0 + attention rounds (V ch+1 inside round ch) ----
            vps = main.enter_context(
                tc.tile_pool(name="vps", bufs=2, space="PSUM"))

            def emit_v_nb(ch, nb):
                vp = vps.tile([P, H, C], F32, tag="vp")
                sb = ch * 4 + nb
                for db in range(DB):
                    nc.tensor.matmul(
                        vp[:], xT[:, db, sb * P:(sb + 1) * P],
                        wkv_v[:, db, :],
                        start=(db == 0), stop=(db == DB - 1))
                nc.vector.tensor_copy(Vst[:, sb, :, :C], vp[:])

            for nb in range(4):
                emit_v_nb(0, nb)

            qkps = main.enter_context(
                tc.tile_pool(name="qkps", bufs=2, space="PSUM"))
            pvps = main.enter_context(
                tc.tile_pool(name="pvps", bufs=2, space="PSUM"))
            ep = main.enter_context(tc.tile_pool(name="ep", bufs=6))

            for ch in range(CH):
                for hg in range(HG):
                    h, g = hg // G, hg % G
                    off = (h % 2) * C
                    kt_jb = h // 2
                    qt_jb = g * 4 + h // 2
                    es = []
                    for gr in range(2):
                        qk = qkps.tile([P, 2, NQ], F32, tag="qk")
                        for s2 in range(2):
                            sb = ch * 4 + gr * 2 + s2
                            nc.tensor.matmul(
                                qk[:, s2, :],
                                KT[off:off + C, kt_jb, sb * P:(sb + 1) * P],
                                QT[off:off + C, qt_jb, :],
                                start=True, stop=True)
                        et = ep.tile([P, 2, NQ], BF16, tag="E")
                        nc.scalar.activation(et[:], qk[:], AF.Exp, scale=SCALE)
                        es.append(et)
                    # pv padded to exactly one 2KB psum bank: matmul start
                    # zeroes the whole 2KB zero-region, so the four qb chains
                    # share one start (first write) and one stop (last write)
                    pv = pvps.tile([P, QB, P], F32, tag="pv")
                    for qb in range(QB):
                        for gr in range(2):
                            for s2 in range(2):
                                sb4 = gr * 2 + s2
                                nc.tensor.matmul(
                                    pv[:, qb, :C + 1],
                                    es[gr][:, s2, qb * P:(qb + 1) * P],
                                    Vst[:, ch * 4 + sb4, h, :],
                                    start=(qb == 0 and sb4 == 0),
                                    stop=(qb == QB - 1 and sb4 == 3))
                    pair, gp = h * 2 + g // 2, g % 2
                    dst = Oacc[:, :, pair, gp, :]
                    if ch == 0:
                        nc.vector.tensor_copy(dst, pv[:, :, :C + 1])
                    else:
                        nc.vector.tensor_add(dst, dst, pv[:, :, :C + 1])
                    # spread next V chunk's projection through this round
                    if ch < CH - 1 and hg % 8 == 7:
                        emit_v_nb(ch + 1, hg // 8)

            if dbg:
                nc.sync.dma_start(dbg_t["dQT"][:], QT[:])
                nc.sync.dma_start(dbg_t["dKT"][:], KT[:])
                nc.sync.dma_start(dbg_t["dVst"][:], Vst[:])
                nc.sync.dma_start(dbg_t["dOacc"][:], Oacc[:])

        # ---- O evacuation: 1/denom, transpose to OT[j, q] ----
        with ExitStack() as tail:
            bpb = tail.enter_context(tc.tile_pool(name="bpbp", bufs=1)).tile(
                [P, D], F32, tag="bpb")
            nc.sync.dma_start(bpb[:], bp[None, :].to_broadcast((P, D)))
            OT = tail.enter_context(tc.tile_pool(name="OTp", bufs=1)).tile(
                [P, DB, NQ], BF16, tag="OT")
            rp = tail.enter_context(tc.tile_pool(name="rp", bufs=1))
            rec = rp.tile([P, QB, H * G // 2, 2], F32, tag="rec")
            nc.vector.reciprocal(rec[:], Oacc[:, :, :, :, C])
            otp = tail.enter_context(tc.tile_pool(name="otp", bufs=3))
            trps = tail.enter_context(
                tc.tile_pool(name="trps", bufs=2, space="PSUM"))
            for pair in range(DB):
                # trp padded to one 2KB bank; single start/stop per bank
                trp = trps.tile([P, 2 * QB, P], BF16, tag="trp")
                for qb in range(QB):
                    ot = otp.tile([P, 2, C], BF16, tag="ot")
                    nc.vector.tensor_mul(
                        ot[:], Oacc[:, qb, pair, :, :C],
                        rec[:, qb, pair, :, None].to_broadcast((P, 2, C)))
                    nc.tensor.matmul(trp[:, qb, :], ot[:], identb[:],
                                     is_transpose=True,
                                     start=(qb == 0), stop=(qb == QB - 1))
                nc.vector.tensor_copy(OT[:, pair, :], trp[:, :QB, :])

            # ---- output projection + bias ----
            wpp = tail.enter_context(tc.tile_pool(name="wpp", bufs=2))
            ops = tail.enter_context(
                tc.tile_pool(name="ops", bufs=3, space="PSUM"))
            osbp = tail.enter_context(tc.tile_pool(name="osbp", bufs=3))
            for ob in range(4):
                wt = wpp.tile([P, DB, NQ], BF16, tag="wp")
                for jb in range(DB):
                    eng = nc.sync if jb % 2 == 0 else nc.gpsimd
                    eng.dma_start(wt[:, jb, :], wp[jb, :, ob, :])
                for qb in range(QB):
                    op = ops.tile([P, NQ], F32, tag="op")
                    for jb in range(DB):
                        nc.tensor.matmul(
                            op[:], OT[:, jb, qb * P:(qb + 1) * P],
                            wt[:, jb, :],
                            start=(jb == 0), stop=(jb == DB - 1))
                    osb = osbp.tile([P, NQ], BF16, tag="osb")
                    nc.vector.tensor_add(osb[:], op[:],
                                         bpb[:, ob * NQ:(ob + 1) * NQ])
                    nc.sync.dma_start(out[qb, :, ob, :], osb[:])
            if dbg:
                nc.sync.dma_start(dbg_t["dOT"][:], OT[:])

    nc.compile()
    return nc


_nc_cache = None


def _prep_inputs(x, Wq, Wkv, Wp, bp):
    """Host-side layout prep (bf16 casts, transposes, reshapes)."""
    import ml_dtypes
    bf16 = ml_dtypes.bfloat16
    x = np.asarray(x, dtype=np.float32)
    # Wq columns to g-major head order: j' = g*512 + h*64 + c, then split
    # into 4 column-chunks of 512 and block rows by 128 for 1KB-line DMAs.
    Wq = (np.asarray(Wq, dtype=np.float32)
          .reshape(D, H, G, C).transpose(0, 2, 1, 3).reshape(D, D))
    wq_p = np.ascontiguousarray(
        Wq.reshape(DB, P, 4, NQ).transpose(2, 0, 1, 3)).astype(bf16)
    wkv_p = np.ascontiguousarray(
        np.asarray(Wkv, dtype=np.float32).reshape(DB, P, 2, NQ)).astype(bf16)
    wp_p = np.ascontiguousarray(
        np.asarray(Wp, dtype=np.float32).reshape(DB, P, 4, NQ)).astype(bf16)
    bp_p = np.ascontiguousarray(np.asarray(bp, dtype=np.float32))
    # x^T per batch: [d, n] -> [DB, P, N]
    xts = [np.ascontiguousarray(x[b].T).astype(bf16).reshape(DB, P, N)
           for b in range(B)]
    return xts, wq_p, wkv_p, wp_p, bp_p


def make_in_maps(x, Wq, Wkv, Wp, bp):
    xts, wq_p, wkv_p, wp_p, bp_p = _prep_inputs(x, Wq, Wkv, Wp, bp)
    in_maps = []
    for c in range(8):
        b, qc = c // 4, c % 4
        xqt = np.ascontiguousarray(
            xts[b].reshape(D, N)[:, qc * NQ:(qc + 1) * NQ]).reshape(DB, P, NQ)
        in_maps.append({
            "xt": xts[b], "xqt": xqt,
            "wq": wq_p, "wkv": wkv_p, "wp": wp_p, "bp": bp_p,
        })
    return in_maps


def kernel(x, Wq, Wkv, Wp, bp):
    global _nc_cache
    if _nc_cache is None:
        _nc_cache = build_program()
    nc = _nc_cache
    in_maps = make_in_maps(x, Wq, Wkv, Wp, bp)
    res = run_bass_kernel_spmd(nc, in_maps, list(range(8)))
    outp = np.empty((B, N, D), np.float32)
    for c in range(8):
        b, qc = c // 4, c % 4
        o = np.asarray(res.results[c]["out"], dtype=np.float32)
        outp[b, qc * NQ:(qc + 1) * NQ] = o.transpose(0, 1, 2, 3).reshape(
            QB, P, D).reshape(NQ, D)
    return outp


# revision 37
# speedup vs baseline: 1.4779x; 1.1051x over previous
"""Trainium2 Bass kernel for GroupedQuerySelfAttention (v2, restructured).

Problem: B=2, N=2048, D=2048, H=8 kv-heads, G=4 (32 query heads), C=64.
  q = (x @ Wq) / sqrt(32);  kv = x @ Wkv;  k, v = split(kv)
  per (b, h, g): S = Qg K^T;  A = softmax(S);  O = A V
  out = concat_heads(O) @ Wp + bp

Sharding: 8 cores = 2 batches x 4 query-chunks of 512 rows. Each core
computes K/V for its whole batch (duplicated within the 4-core group --
collectives are slower than the duplicated compute under this machine's
cost model), attention for its 512 query rows over all 32 heads, and its
512 rows of the output projection. Host concatenates.

Key layout choices (all matmul inputs bf16; psum f32):
  xT   [d, n]   host-pre-transposed x, so no PE transposes of x
  Q^T  [j, n]   g-major head order (host-permuted Wq) so Q^T and K^T
                partition offsets line up per (h, g)
  K^T  [j, n]
  V~   [s, h, 65]  V columns + ones column (softmax denominators fall
                out of the PV matmul for free)
  S^T  [s, q]   lhsT = K^T slice, rhs = Q^T slice (contraction c=64)
  E^T  = exp(S^T / sqrt(32)) -> bf16, exp'd in [128, 2, 512] groups
  PV:  out O[q, 65] with lhsT = E^T (q-partition output: free dim is
                only 65, halving PE cost vs the [65, q] orientation)
  O accumulated over seq chunks in SBUF f32; per-row 1/denom applied at
  the end; O transposed back to [j, q] via PE for the out projection.

Pipeline: K proj -> Q proj -> V chunk0 -> 4 attention rounds (one per
seq chunk; V chunk ch+1 is projected inside round ch, hidden under the
ACT-engine exp stream which is the phase bottleneck) -> O evac -> out
projection.
"""

import numpy as np
from contextlib import ExitStack

import concourse.bass as bass
import concourse.tile as tile
from concourse import bacc, mybir
from concourse.bass_utils import run_bass_kernel_spmd
from concourse.masks import make_identity

P = 128
B, N, D = 2, 2048, 2048
H, G, C = 8, 4, 64
HG = H * G                     # 32 query heads
NQ = 512                       # query rows per core
DB = D // P                    # 16 d-blocks
NB = N // P                    # 16 seq blocks
QB = NQ // P                   # 4 query blocks
CH = N // NQ                   # 4 seq chunks
SCALE = float(1.0 / np.sqrt(HG))
WARMUP = 200
F32 = mybir.dt.float32
BF16 = mybir.dt.bfloat16
AF = mybir.ActivationFunctionType


def build_program(n_cores=8, dbg=False, upto=99):
    nc = bacc.Bacc("TRN2", target_bir_lowering=False, debug=False,
                   num_devices=n_cores)
    dbg_t = {}
    if dbg:
        for nm, shp in [("dQT", [P, DB, NQ]), ("dKT", [P, 4, N]),
                        ("dVst", [P, NB, H, C + 1]), ("dOT", [P, DB, NQ])]:
            dbg_t[nm] = nc.dram_tensor(nm, shp, BF16, kind="ExternalOutput").ap()
        dbg_t["dOacc"] = nc.dram_tensor(
            "dOacc", [P, QB, HG // 2, 2, C + 1], F32, kind="ExternalOutput").ap()
    # host-prepared partition-major layouts (see _prep_inputs below)
    xt = nc.dram_tensor("xt", [P, DB, N], BF16, kind="ExternalInput").ap()
    wq = nc.dram_tensor("wq", [4, P, DB, NQ], BF16, kind="ExternalInput").ap()
    wkv = nc.dram_tensor("wkv", [P, DB, 2, NQ], BF16, kind="ExternalInput").ap()
    wp = nc.dram_tensor("wp", [P, DB, 4, NQ], BF16, kind="ExternalInput").ap()
    bp = nc.dram_tensor("bp", [D], F32, kind="ExternalInput").ap()
    out = nc.dram_tensor("out", [QB, P, 4, NQ], BF16, kind="ExternalOutput").ap()

    with tile.TileContext(nc) as tc, ExitStack() as top:
        per = top.enter_context(tc.tile_pool(name="per", bufs=1))
        identb = per.tile([P, P], BF16, tag="identb")
        make_identity(nc, identb[:])
        ones = per.tile([P, 1], BF16, tag="ones")
        nc.gpsimd.memset(ones[:], 1.0)
        # O accumulator survives from the attention rounds into the tail
        Oacc = top.enter_context(tc.tile_pool(name="Oaccp", bufs=1)).tile(
            [P, QB, HG // 2, 2, C + 1], F32, tag="Oacc")

        with ExitStack() as main:
            QT = main.enter_context(tc.tile_pool(name="QTp", bufs=1)).tile(
                [P, DB, NQ], BF16, tag="QT")
            KT = main.enter_context(tc.tile_pool(name="KTp", bufs=1)).tile(
                [P, H * C // P, N], BF16, tag="KT")
            Vst = main.enter_context(tc.tile_pool(name="Vstp", bufs=1)).tile(
                [P, NB, H, C + 1], BF16, tag="Vst")
            nc.vector.tensor_copy(
                Vst[:, :, :, C:C + 1],
                ones[:, None, None, :].to_broadcast((P, NB, H, 1)))
            ep = main.enter_context(tc.tile_pool(name="ep", bufs=6))

            # ---------------- attention round bodies ----------------
            def emit_round(ch, qkps, pvps):
                for hg in range(HG):
                    h, g = hg // G, hg % G
                    off = (h % 2) * C
                    kt_jb = h // 2
                    qt_jb = g * 4 + h // 2
                    es = []
                    for gr in range(2):
                        qk = qkps.tile([P, 2, NQ], F32, tag="qk")
                        for s2 in range(2):
                            sb = ch * 4 + gr * 2 + s2
                            nc.tensor.matmul(
                                qk[:, s2, :],
                                KT[off:off + C, kt_jb, sb * P:(sb + 1) * P],
                                QT[off:off + C, qt_jb, :],
                                start=True, stop=True)
                        et = ep.tile([P, 2, NQ], BF16, tag="E")
                        nc.scalar.activation(et[:], qk[:], AF.Exp, scale=SCALE)
                        es.append(et)
                    # pv padded to exactly one 2KB psum bank: matmul start
                    # zeroes the whole 2KB zero-region, so the four qb chains
                    # share one start (first write) and one stop (last write)
                    pv = pvps.tile([P, QB, P], F32, tag="pv")
                    for qb in range(QB):
                        for gr in range(2):
                            for s2 in range(2):
                                sb4 = gr * 2 + s2
                                nc.tensor.matmul(
                                    pv[:, qb, :C + 1],
                                    es[gr][:, s2, qb * P:(qb + 1) * P],
                                    Vst[:, ch * 4 + sb4, h, :],
                                    start=(qb == 0 and sb4 == 0),
                                    stop=(qb == QB - 1 and sb4 == 3))
                    pair, gp = h * 2 + g // 2, g % 2
                    dst = Oacc[:, :, pair, gp, :]
                    if ch == 0:
                        nc.vector.tensor_copy(dst, pv[:, :, :C + 1])
                    else:
                        nc.vector.tensor_add(dst, dst, pv[:, :, :C + 1])
                    yield hg

            with ExitStack() as vscope:
                # DMA order matters: the cost model serializes all DMAs on
                # one shared device, so K-critical tiles go first and xT
                # arrives n-chunk by n-chunk as the K chains consume it
                xts = vscope.enter_context(tc.tile_pool(name="xts", bufs=1))
                xT = xts.tile([P, DB, N], BF16, tag="xT")
                wkvp = vscope.enter_context(tc.tile_pool(name="wkvp", bufs=1))
                wkv_v = wkvp.tile([P, DB, NQ], BF16, tag="wkv_v")
                # wq stream buffers live beside wkv_k (not reusing its SBUF)
                # so the wq transfers are not WAR-serialized behind K's
                # last matmul
                wqp = vscope.enter_context(tc.tile_pool(name="wqp", bufs=2))

                # ---- K projection: K^T[j, n] for all 4 chunks ----
                with ExitStack() as s:
                    wkp = s.enter_context(tc.tile_pool(name="wkp", bufs=1))
                    wkv_k = wkp.tile([P, DB, NQ], BF16, tag="wkv_k")
                    nc.sync.dma_start(wkv_k[:, 0:8, :], wkv[:, 0:8, 0, :])
                    nc.sync.dma_start(xT[:, :, 0:NQ], xt[:, :, 0:NQ])
                    nc.scalar.dma_start(wkv_k[:, 8:16, :], wkv[:, 8:16, 0, :])
                    for ch in range(1, CH):
                        eng = nc.sync if ch % 2 == 0 else nc.scalar
                        eng.dma_start(xT[:, :, ch * NQ:(ch + 1) * NQ],
                                      xt[:, :, ch * NQ:(ch + 1) * NQ])
                    for hf in range(2):
                        nc.gpsimd.dma_start(wkv_v[:, hf * 8:(hf + 1) * 8, :],
                                            wkv[:, hf * 8:(hf + 1) * 8, 1, :])
                    # PE warmup: keep a busy streak from t=0 so the p-state
                    # ramp reaches full clock before the first real matmul
                    wups = s.enter_context(
                        tc.tile_pool(name="wups", bufs=1, space="PSUM"))
                    wup = wups.tile([P, P], BF16, tag="wup")
                    for _ in range(WARMUP):
                        nc.tensor.matmul(wup[:], identb[:], identb[:],
                                         is_transpose=True,
                                         start=True, stop=True)
                    kps = s.enter_context(
                        tc.tile_pool(name="kps", bufs=4, space="PSUM"))
                    for ch in range(CH):
                        for jb in range(4):
                            kp = kps.tile([P, NQ], F32, tag="kp")
                            for db in range(DB):
                                nc.tensor.matmul(
                                    kp[:], wkv_k[:, db, jb * P:(jb + 1) * P],
                                    xT[:, db, ch * NQ:(ch + 1) * NQ],
                                    start=(db == 0), stop=(db == DB - 1))
                            nc.vector.tensor_copy(
                                KT[:, jb, ch * NQ:(ch + 1) * NQ], kp[:])

                # ---- Q projection: Q^T[j, n] for this core's 512 rows ----
                with ExitStack() as s:
                  if upto >= 2:
                    qps = s.enter_context(
                        tc.tile_pool(name="qps", bufs=4, space="PSUM"))
                    for jc in range(4):
                        wts = []
                        for q4 in range(4):
                            wt = wqp.tile([P, 4, NQ], BF16, tag="wq")
                            eng = nc.sync if q4 % 2 == 0 else nc.scalar
                            eng.dma_start(wt[:],
                                          wq[jc, :, q4 * 4:(q4 + 1) * 4, :])
                            wts.append(wt)
                        qp = [qps.tile([P, NQ], F32, tag="qp",
                                       name=f"qp{jc}_{j}") for j in range(4)]
                        for db in range(DB):
                            for jb in range(4):
                                nc.tensor.matmul(
                                    qp[jb][:],
                                    wts[db // 4][:, db % 4, jb * P:(jb + 1) * P],
                                    xT[:, db, 0:NQ],
                                    start=(db == 0), stop=(db == DB - 1))
                        for jb in range(4):
                            nc.vector.tensor_copy(QT[:, jc * 4 + jb, :],
                                                  qp[jb][:])

                # ---- V chunk 0 (own pool) ----
                def emit_v_nb(pool, ch, nb):
                    vp = pool.tile([P, H, C], F32, tag="vp")
                    sb = ch * 4 + nb
                    for db in range(DB):
                        nc.tensor.matmul(
                            vp[:], xT[:, db, sb * P:(sb + 1) * P],
                            wkv_v[:, db, :],
                            start=(db == 0), stop=(db == DB - 1))
                    nc.vector.tensor_copy(Vst[:, sb, :, :C], vp[:])

                if upto >= 3:
                    with ExitStack() as s:
                        vps0 = s.enter_context(
                            tc.tile_pool(name="vps0", bufs=2, space="PSUM"))
                        for nb in range(4):
                            emit_v_nb(vps0, 0, nb)

                # ---- rounds 0..2 with V chunk ch+1 spread through round ch
                if upto >= 4:
                    qkpsA = vscope.enter_context(
                        tc.tile_pool(name="qkpsA", bufs=3, space="PSUM"))
                    pvpsA = vscope.enter_context(
                        tc.tile_pool(name="pvpsA", bufs=1, space="PSUM"))
                    vps = vscope.enter_context(
                        tc.tile_pool(name="vps", bufs=1, space="PSUM"))
                    for ch in range(CH - 1):
                        for hg in emit_round(ch, qkpsA, pvpsA):
                            if hg % 8 == 7:
                                emit_v_nb(vps, ch + 1, hg // 8)
            # xT / wkv_v / vps freed here: round 3 + interleaved O evac

            if upto >= 5:
                OT = main.enter_context(tc.tile_pool(name="OTp", bufs=1)).tile(
                    [P, DB, NQ], BF16, tag="OT")
                rp = main.enter_context(tc.tile_pool(name="rp", bufs=1))
                rec = rp.tile([P, QB, HG // 2, 2], F32, tag="rec")
                otp = main.enter_context(tc.tile_pool(name="otp", bufs=3))
                r3 = main.enter_context(ExitStack())
                qkpsB = r3.enter_context(
                    tc.tile_pool(name="qkpsB", bufs=3, space="PSUM"))
                pvpsB = r3.enter_context(
                    tc.tile_pool(name="pvpsB", bufs=1, space="PSUM"))

                def emit_evac(pair):
                    nc.vector.reciprocal(rec[:, :, pair, :],
                                         Oacc[:, :, pair, :, C])
                    # trp shares the pv bank pool (one 2KB bank per tile)
                    trp = pvpsB.tile([P, 2 * QB, P], BF16, tag="trp")
                    for qb in range(QB):
                        ot = otp.tile([P, 2, C], BF16, tag="ot")
                        nc.vector.tensor_mul(
                            ot[:], Oacc[:, qb, pair, :, :C],
                            rec[:, qb, pair, :, None].to_broadcast((P, 2, C)))
                        nc.tensor.matmul(trp[:, qb, :], ot[:], identb[:],
                                         is_transpose=True,
                                         start=(qb == 0), stop=(qb == QB - 1))
                    nc.vector.tensor_copy(OT[:, pair, :], trp[:, :QB, :])

                for hg in emit_round(CH - 1, qkpsB, pvpsB):
                    if hg % 2 == 1:          # (h, g=1) -> pair 2h; (h, g=3) -> 2h+1
                        h, g = hg // G, hg % G
                        emit_evac(h * 2 + g // 2)
                r3.close()

            if dbg:
                nc.sync.dma_start(dbg_t["dQT"][:], QT[:])
                nc.sync.dma_start(dbg_t["dKT"][:], KT[:])
                nc.sync.dma_start(dbg_t["dVst"][:], Vst[:])
                nc.sync.dma_start(dbg_t["dOacc"][:], Oacc[:])
                if upto >= 5:
                    nc.sync.dma_start(dbg_t["dOT"][:], OT[:])

            # ---- output projection + bias ----
            if upto >= 6:
                bpb = main.enter_context(
                    tc.tile_pool(name="bpbp", bufs=1)).tile(
                        [P, D], F32, tag="bpb")
                nc.sync.dma_start(bpb[:], bp[None, :].to_broadcast((P, D)))
                wpp = main.enter_context(tc.tile_pool(name="wpp", bufs=3))
                ops = main.enter_context(
                    tc.tile_pool(name="ops", bufs=3, space="PSUM"))
                osbp = main.enter_context(tc.tile_pool(name="osbp", bufs=3))
                for ob in range(4):
                    wts = []
                    for hf in range(2):
                        wt = wpp.tile([P, 8, NQ], BF16, tag="wph")
                        eng = nc.sync if hf == 0 else nc.scalar
                        eng.dma_start(wt[:],
                                      wp[:, hf * 8:(hf + 1) * 8, ob, :])
                        wts.append(wt)
                    for qb in range(QB):
                        op = ops.tile([P, NQ], F32, tag="op")
                        for jb in range(DB):
                            nc.tensor.matmul(
                                op[:], OT[:, jb, qb * P:(qb + 1) * P],
                                wts[jb // 8][:, jb % 8, :],
                                start=(jb == 0), stop=(jb == DB - 1))
                        osb = osbp.tile([P, NQ], BF16, tag="osb")
                        nc.vector.tensor_add(osb[:], op[:],
                                             bpb[:, ob * NQ:(ob + 1) * NQ])
                        nc.sync.dma_start(out[qb, :, ob, :], osb[:])

    nc.compile()
    return nc


_nc_cache = None


def _prep_inputs(x, Wq, Wkv, Wp, bp):
    """Host-side layout prep (bf16 casts, transposes, reshapes)."""
    import ml_dtypes
    bf16 = ml_dtypes.bfloat16
    x = np.asarray(x, dtype=np.float32)
    # Wq columns to g-major head order: j' = g*512 + h*64 + c, then to
    # partition-major [jc, p, db, j] so each jc-chunk is 1-2 big DMAs.
    Wq = (np.asarray(Wq, dtype=np.float32)
          .reshape(D, H, G, C).transpose(0, 2, 1, 3).reshape(D, D))
    wq_p = np.ascontiguousarray(
        Wq.reshape(DB, P, 4, NQ).transpose(2, 1, 0, 3)).astype(bf16)
    wkv_p = np.ascontiguousarray(
        np.asarray(Wkv, dtype=np.float32)
        .reshape(DB, P, 2, NQ).transpose(1, 0, 2, 3)).astype(bf16)
    wp_p = np.ascontiguousarray(
        np.asarray(Wp, dtype=np.float32)
        .reshape(DB, P, 4, NQ).transpose(1, 0, 2, 3)).astype(bf16)
    bp_p = np.ascontiguousarray(np.asarray(bp, dtype=np.float32))
    # x^T per batch: [d, n] -> partition-major [P, DB, N]
    xts = [np.ascontiguousarray(
               x[b].T.reshape(DB, P, N).transpose(1, 0, 2)).astype(bf16)
           for b in range(B)]
    return xts, wq_p, wkv_p, wp_p, bp_p


def make_in_maps(x, Wq, Wkv, Wp, bp):
    xts, wq_p, wkv_p, wp_p, bp_p = _prep_inputs(x, Wq, Wkv, Wp, bp)
    in_maps = []
    for c in range(8):
        b, qc = c // 4, c % 4
        # rotate the sequence axis so this core's query chunk is at n=0;
        # attention is invariant to a consistent permutation of the k/v axis
        xt_c = np.ascontiguousarray(np.roll(xts[b], -qc * NQ, axis=2))
        in_maps.append({
            "xt": xt_c,
            "wq": wq_p, "wkv": wkv_p, "wp": wp_p, "bp": bp_p,
        })
    return in_maps


def kernel(x, Wq, Wkv, Wp, bp):
    global _nc_cache
    if _nc_cache is None:
        _nc_cache = build_program()
    nc = _nc_cache
    in_maps = make_in_maps(x, Wq, Wkv, Wp, bp)
    res = run_bass_kernel_spmd(nc, in_maps, list(range(8)))
    outp = np.empty((B, N, D), np.float32)
    for c in range(8):
        b, qc = c // 4, c % 4
        o = np.asarray(res.results[c]["out"], dtype=np.float32)
        outp[b, qc * NQ:(qc + 1) * NQ] = o.transpose(0, 1, 2, 3).reshape(
            QB, P, D).reshape(NQ, D)
    return outp


# revision 48
# speedup vs baseline: 1.4996x; 1.0147x over previous
"""Trainium2 Bass kernel for GroupedQuerySelfAttention (v2, restructured).

Problem: B=2, N=2048, D=2048, H=8 kv-heads, G=4 (32 query heads), C=64.
  q = (x @ Wq) / sqrt(32);  kv = x @ Wkv;  k, v = split(kv)
  per (b, h, g): S = Qg K^T;  A = softmax(S);  O = A V
  out = concat_heads(O) @ Wp + bp

Sharding: 8 cores = 2 batches x 4 query-chunks of 512 rows. Each core
computes K/V for its whole batch (duplicated within the 4-core group --
collectives are slower than the duplicated compute under this machine's
cost model), attention for its 512 query rows over all 32 heads, and its
512 rows of the output projection. Host concatenates.

Key layout choices (all matmul inputs bf16; psum f32):
  xT   [d, n]   host-pre-transposed x, so no PE transposes of x
  Q^T  [j, n]   g-major head order (host-permuted Wq) so Q^T and K^T
                partition offsets line up per (h, g)
  K^T  [j, n]
  V~   [s, h, 65]  V columns + ones column (softmax denominators fall
                out of the PV matmul for free)
  S^T  [s, q]   lhsT = K^T slice, rhs = Q^T slice (contraction c=64)
  E^T  = exp(S^T / sqrt(32)) -> bf16, exp'd in [128, 2, 512] groups
  PV:  out O[q, 65] with lhsT = E^T (q-partition output: free dim is
                only 65, halving PE cost vs the [65, q] orientation)
  O accumulated over seq chunks in SBUF f32; per-row 1/denom applied at
  the end; O transposed back to [j, q] via PE for the out projection.

Pipeline (one TileContext; the Tile scheduler overlaps across phase
boundaries):
  1. K projection (PE warmed up from t=0 with dummy transposes so the
     p-state ramp hits full clock before the first real matmul; DMAs
     ordered so K-critical tiles land first -- the cost model serializes
     all DMAs on one shared device).
  2. V chunk 0.
  3. Round 0 merged with the Q projection (jc == g: once Q-chunk g is
     projected, all heads with that g run their round-0 QK/exp/PV), so
     the ACT exp stream starts ~50us early.  V chunk 1 projected here.
  4. Rounds 1-2, ACT-bound at the exp floor: QK/exp run 2 head-groups
     ahead of PV so the in-order PE never parks a PV (which waits on
     exp) ahead of an independent QK; V chunks 2-3 drip in 2 matmuls
     per head-group.
  5. Round 3 with the O evacuation (recip + PE transpose into OT)
     trailing per head-pair.
  6. Output projection + bias, bf16 out (host casts back to f32).

Gotchas encoded here: matmul start=True zeroes the whole 2KB psum
zero-region, so multi-chain psum tiles use one start/stop per bank;
SBUF pool reuse creates WAR serialization, so streamed weights get
pools disjoint from the tensors the previous phase still reads.
"""

import numpy as np
from contextlib import ExitStack

import concourse.bass as bass
import concourse.tile as tile
from concourse import bacc, mybir
from concourse.bass_utils import run_bass_kernel_spmd
from concourse.masks import make_identity

P = 128
B, N, D = 2, 2048, 2048
H, G, C = 8, 4, 64
HG = H * G                     # 32 query heads
NQ = 512                       # query rows per core
DB = D // P                    # 16 d-blocks
NB = N // P                    # 16 seq blocks
QB = NQ // P                   # 4 query blocks
CH = N // NQ                   # 4 seq chunks
SCALE = float(1.0 / np.sqrt(HG))
WARMUP = 200
F32 = mybir.dt.float32
BF16 = mybir.dt.bfloat16
AF = mybir.ActivationFunctionType


def build_program(n_cores=8, dbg=False, upto=99):
    nc = bacc.Bacc("TRN2", target_bir_lowering=False, debug=False,
                   num_devices=n_cores)
    dbg_t = {}
    if dbg:
        for nm, shp in [("dQT", [P, DB, NQ]), ("dKT", [P, 4, N]),
                        ("dVst", [P, NB, H, C + 1]), ("dOT", [P, DB, NQ])]:
            dbg_t[nm] = nc.dram_tensor(nm, shp, BF16, kind="ExternalOutput").ap()
        dbg_t["dOacc"] = nc.dram_tensor(
            "dOacc", [P, QB, HG // 2, 2, C + 1], F32, kind="ExternalOutput").ap()
    # host-prepared partition-major layouts (see _prep_inputs below)
    xt = nc.dram_tensor("xt", [P, DB, N], BF16, kind="ExternalInput").ap()
    wq = nc.dram_tensor("wq", [4, P, DB, NQ], BF16, kind="ExternalInput").ap()
    wkv = nc.dram_tensor("wkv", [P, DB, 2, NQ], BF16, kind="ExternalInput").ap()
    wp = nc.dram_tensor("wp", [P, DB, 4, NQ], BF16, kind="ExternalInput").ap()
    bp = nc.dram_tensor("bp", [D], F32, kind="ExternalInput").ap()
    out = nc.dram_tensor("out", [QB, P, 4, NQ], BF16, kind="ExternalOutput").ap()

    with tile.TileContext(nc) as tc, ExitStack() as top:
        per = top.enter_context(tc.tile_pool(name="per", bufs=1))
        identb = per.tile([P, P], BF16, tag="identb")
        make_identity(nc, identb[:])
        ones = per.tile([P, 1], BF16, tag="ones")
        nc.gpsimd.memset(ones[:], 1.0)
        # O accumulator survives from the attention rounds into the tail
        Oacc = top.enter_context(tc.tile_pool(name="Oaccp", bufs=1)).tile(
            [P, QB, HG // 2, 2, C + 1], F32, tag="Oacc")

        with ExitStack() as main:
            QT = main.enter_context(tc.tile_pool(name="QTp", bufs=1)).tile(
                [P, DB, NQ], BF16, tag="QT")
            KT = main.enter_context(tc.tile_pool(name="KTp", bufs=1)).tile(
                [P, H * C // P, N], BF16, tag="KT")
            Vst = main.enter_context(tc.tile_pool(name="Vstp", bufs=1)).tile(
                [P, NB, H, C + 1], BF16, tag="Vst")
            nc.vector.tensor_copy(
                Vst[:, :, :, C:C + 1],
                ones[:, None, None, :].to_broadcast((P, NB, H, 1)))
            ep = main.enter_context(tc.tile_pool(name="ep", bufs=6))

            # ---------------- attention round bodies ----------------
            # Software-pipelined two hg deep: QK/exp of hg+1, hg+2 are
            # emitted before PV of hg, so the in-order PE never queues a PV
            # (which waits on its exp) ahead of the next independent QK --
            # that ordering would put a ~1.4us bubble in the ACT exp stream
            # per head group.  QK psum tiles are 3 banks and exp'd in one
            # free-1536 activation (sb-groups stream across hg boundaries)
            # to amortize the ACT per-instruction overhead.
            class QkStream:
                """Streams QK sb-tiles into 3-bank psum groups, exp'd as
                one ACT instruction each; slots[] maps (ch,h,g,sb4) to the
                bf16 E tile + slot the PV matmuls read from."""
                W = 2

                def __init__(self, qkps):
                    self.qkps = qkps
                    self.tile = None
                    self.entries = []
                    self.slots = {}

                def push(self, ch, h, g, sb4):
                    off = (h % 2) * C
                    if self.tile is None:
                        self.tile = self.qkps.tile([P, self.W, NQ], F32,
                                                   tag="qk")
                    slot = len(self.entries)
                    sb = ch * 4 + sb4
                    nc.tensor.matmul(
                        self.tile[:, slot, :],
                        KT[off:off + C, h // 2, sb * P:(sb + 1) * P],
                        QT[off:off + C, g * 4 + h // 2, :],
                        start=True, stop=True)
                    self.entries.append((ch, h, g, sb4))
                    if len(self.entries) == self.W:
                        self.flush()

                def flush(self):
                    if self.tile is None:
                        return
                    n = len(self.entries)
                    et = ep.tile([P, self.W, NQ], BF16, tag="E")
                    nc.scalar.activation(et[:, :n, :], self.tile[:, :n, :],
                                         AF.Exp, scale=SCALE)
                    for i, key in enumerate(self.entries):
                        self.slots[key] = (et, i)
                    self.tile = None
                    self.entries = []

            def emit_qk_exp(ch, h, g, stream):
                for sb4 in range(4):
                    stream.push(ch, h, g, sb4)
                return stream

            def emit_pv(ch, h, g, stream, pvps):
                # pv padded to exactly one 2KB psum bank: matmul start
                # zeroes the whole 2KB zero-region, so the four qb chains
                # share one start (first write) and one stop (last write)
                pv = pvps.tile([P, QB, P], F32, tag="pv")
                for qb in range(QB):
                    for sb4 in range(4):
                        et, slot = stream.slots[(ch, h, g, sb4)]
                        nc.tensor.matmul(
                            pv[:, qb, :C + 1],
                            et[:, slot, qb * P:(qb + 1) * P],
                            Vst[:, ch * 4 + sb4, h, :],
                            start=(qb == 0 and sb4 == 0),
                            stop=(qb == QB - 1 and sb4 == 3))
                for sb4 in range(4):
                    del stream.slots[(ch, h, g, sb4)]
                pair, gp = h * 2 + g // 2, g % 2
                dst = Oacc[:, :, pair, gp, :]
                if ch == 0:
                    nc.vector.tensor_copy(dst, pv[:, :, :C + 1])
                else:
                    nc.vector.tensor_add(dst, dst, pv[:, :, :C + 1])

            def emit_round(ch, stream, pvps, pend):
                """Emit one round 2-deep pipelined; pend is a shared deque
                of (ch, h, g, stream) whose PV has not been emitted yet.
                Yields (qk_hg, pv_hg_or_None) after each step."""
                for hg in range(HG):
                    h, g = hg // G, hg % G
                    pend.append((ch, h, g, emit_qk_exp(ch, h, g, stream)))
                    done = None
                    if len(pend) > 2:
                        e = pend.pop(0)
                        emit_pv(*e, pvps)
                        done = e[1] * G + e[2]
                    yield hg, done
                stream.flush()

            def flush_pend(pend, pvps, n=None):
                flushed = []
                while pend and (n is None or len(flushed) < n):
                    e = pend.pop(0)
                    e[3].flush()
                    emit_pv(*e, pvps)
                    flushed.append(e[1] * G + e[2])
                return flushed

            with ExitStack() as vscope:
                # DMA order matters: the cost model serializes all DMAs on
                # one shared device, so K-critical tiles go first and xT
                # arrives n-chunk by n-chunk as the K chains consume it
                xts = vscope.enter_context(tc.tile_pool(name="xts", bufs=1))
                xT = xts.tile([P, DB, N], BF16, tag="xT")
                wkvp = vscope.enter_context(tc.tile_pool(name="wkvp", bufs=1))
                wkv_v = wkvp.tile([P, DB, NQ], BF16, tag="wkv_v")
                # wq stream buffers live beside wkv_k (not reusing its SBUF)
                # so the wq transfers are not WAR-serialized behind K's
                # last matmul
                wqp = vscope.enter_context(tc.tile_pool(name="wqp", bufs=4))

                # ---- K projection: K^T[j, n] for all 4 chunks ----
                with ExitStack() as s:
                    wkp = s.enter_context(tc.tile_pool(name="wkp", bufs=1))
                    wkv_k = wkp.tile([P, DB, NQ], BF16, tag="wkv_k")
                    nc.sync.dma_start(wkv_k[:, 0:8, :], wkv[:, 0:8, 0, :])
                    nc.sync.dma_start(xT[:, :, 0:NQ], xt[:, :, 0:NQ])
                    nc.scalar.dma_start(wkv_k[:, 8:16, :], wkv[:, 8:16, 0, :])
                    for ch in range(1, CH):
                        eng = nc.sync if ch % 2 == 0 else nc.scalar
                        eng.dma_start(xT[:, :, ch * NQ:(ch + 1) * NQ],
                                      xt[:, :, ch * NQ:(ch + 1) * NQ])
                    for hf in range(2):
                        nc.gpsimd.dma_start(wkv_v[:, hf * 8:(hf + 1) * 8, :],
                                            wkv[:, hf * 8:(hf + 1) * 8, 1, :])
                    # PE warmup: keep a busy streak from t=0 so the p-state
                    # ramp reaches full clock before the first real matmul
                    wups = s.enter_context(
                        tc.tile_pool(name="wups", bufs=1, space="PSUM"))
                    wup = wups.tile([P, P], BF16, tag="wup")
                    for _ in range(WARMUP):
                        nc.tensor.matmul(wup[:], identb[:], identb[:],
                                         is_transpose=True,
                                         start=True, stop=True)
                    kps = s.enter_context(
                        tc.tile_pool(name="kps", bufs=4, space="PSUM"))
                    for ch in range(CH):
                        for jb in range(4):
                            kp = kps.tile([P, NQ], F32, tag="kp")
                            for db in range(DB):
                                nc.tensor.matmul(
                                    kp[:], wkv_k[:, db, jb * P:(jb + 1) * P],
                                    xT[:, db, ch * NQ:(ch + 1) * NQ],
                                    start=(db == 0), stop=(db == DB - 1))
                            nc.vector.tensor_copy(
                                KT[:, jb, ch * NQ:(ch + 1) * NQ], kp[:])

                # ---- V projection helper ----
                def emit_v_nb(pool, ch, nb):
                    vp = pool.tile([P, H, C], F32, tag="vp")
                    sb = ch * 4 + nb
                    for db in range(DB):
                        nc.tensor.matmul(
                            vp[:], xT[:, db, sb * P:(sb + 1) * P],
                            wkv_v[:, db, :],
                            start=(db == 0), stop=(db == DB - 1))
                    nc.vector.tensor_copy(Vst[:, sb, :, :C], vp[:])

                # ---- V chunk 0 (before Q so round 0 can consume it) ----
                if upto >= 2:
                    with ExitStack() as s:
                        vps0 = s.enter_context(
                            tc.tile_pool(name="vps0", bufs=2, space="PSUM"))
                        for nb in range(4):
                            emit_v_nb(vps0, 0, nb)

                # ---- merged Q projection + attention round 0 ----
                # jc == g: after Q-chunk jc is projected, all heads with
                # g == jc can run their round-0 QK/exp/PV, so the ACT
                # exp stream starts ~50us earlier.  Q's psum chains share
                # the qk pool tiles (two 1-bank chains per 2-bank tile).
                if upto >= 3:
                    qkpsA = vscope.enter_context(
                        tc.tile_pool(name="qkpsA", bufs=3, space="PSUM"))
                    pvpsA = vscope.enter_context(
                        tc.tile_pool(name="pvpsA", bufs=1, space="PSUM"))
                    vps = vscope.enter_context(
                        tc.tile_pool(name="vps", bufs=1, space="PSUM"))
                    pend = []
                    streamA = QkStream(qkpsA)
                    for jc in range(4):
                        wts = []
                        for q4 in range(4):
                            wt = wqp.tile([P, 4, NQ], BF16, tag="wq")
                            eng = nc.sync if q4 % 2 == 0 else nc.scalar
                            eng.dma_start(wt[:],
                                          wq[jc, :, q4 * 4:(q4 + 1) * 4, :])
                            wts.append(wt)
                        qp = [qkpsA.tile([P, 2, NQ], F32, tag="qk",
                                         name=f"qp{jc}_{j}") for j in range(2)]
                        for db in range(DB):
                            for jb in range(4):
                                nc.tensor.matmul(
                                    qp[jb // 2][:, jb % 2, :],
                                    wts[db // 4][:, db % 4, jb * P:(jb + 1) * P],
                                    xT[:, db, 0:NQ],
                                    start=(db == 0), stop=(db == DB - 1))
                        for jb in range(4):
                            nc.vector.tensor_copy(QT[:, jc * 4 + jb, :],
                                                  qp[jb // 2][:, jb % 2, :])
                        if upto >= 4:
                            g = jc
                            for h in range(H):
                                pend.append((0, h, g,
                                             emit_qk_exp(0, h, g, streamA)))
                                if len(pend) > 2:
                                    e = pend.pop(0)
                                    e[3].flush()
                                    emit_pv(*e, pvpsA)
                                if g >= 2 and h % 4 == 3:
                                    emit_v_nb(vps, 1, (g - 2) * 2 + h // 4)

                # ---- rounds 1..2, V chunk ch+1 drip-fed 2 matmuls per
                # hg so the PE never runs a 3.4us V block that would stall
                # the exp stream
                if upto >= 4:
                    class VStepper:
                        def __init__(self, ch):
                            self.work = [(ch * 4 + nb, db) for nb in range(4)
                                         for db in range(DB)]
                            self.i = 0
                            self.vp = None

                        def step(self, n):
                            for _ in range(n):
                                if self.i >= len(self.work):
                                    return
                                sb, db = self.work[self.i]
                                if db == 0:
                                    self.vp = vps.tile([P, H, C], F32,
                                                       tag="vp")
                                nc.tensor.matmul(
                                    self.vp[:], xT[:, db, sb * P:(sb + 1) * P],
                                    wkv_v[:, db, :],
                                    start=(db == 0), stop=(db == DB - 1))
                                if db == DB - 1:
                                    nc.vector.tensor_copy(
                                        Vst[:, sb, :, :C], self.vp[:])
                                self.i += 1

                    for ch in range(1, CH - 1):
                        vstep = VStepper(ch + 1)
                        for hg, _ in emit_round(ch, streamA, pvpsA, pend):
                            vstep.step(2)
                        vstep.step(DB * 4)
                    # drain the cross-round pipeline before the psum pools
                    # of rounds 0-2 close
                    flush_pend(pend, pvpsA)
            # xT / wkv_v / vps freed here: round 3 + interleaved O evac

            if upto >= 5:
                OT = main.enter_context(tc.tile_pool(name="OTp", bufs=1)).tile(
                    [P, DB, NQ], BF16, tag="OT")
                rp = main.enter_context(tc.tile_pool(name="rp", bufs=1))
                rec = rp.tile([P, QB, HG // 2, 2], F32, tag="rec")
                otp = main.enter_context(tc.tile_pool(name="otp", bufs=3))
                r3 = main.enter_context(ExitStack())
                qkpsB = r3.enter_context(
                    tc.tile_pool(name="qkpsB", bufs=3, space="PSUM"))
                pvpsB = r3.enter_context(
                    tc.tile_pool(name="pvpsB", bufs=1, space="PSUM"))

                def emit_evac(pair):
                    nc.vector.reciprocal(rec[:, :, pair, :],
                                         Oacc[:, :, pair, :, C])
                    # trp shares the pv bank pool (one 2KB bank per tile)
                    trp = pvpsB.tile([P, 2 * QB, P], BF16, tag="trp")
                    for qb in range(QB):
                        ot = otp.tile([P, 2, C], BF16, tag="ot")
                        nc.vector.tensor_mul(
                            ot[:], Oacc[:, qb, pair, :, :C],
                            rec[:, qb, pair, :, None].to_broadcast((P, 2, C)))
                        nc.tensor.matmul(trp[:, qb, :], ot[:], identb[:],
                                         is_transpose=True,
                                         start=(qb == 0), stop=(qb == QB - 1))
                    nc.vector.tensor_copy(OT[:, pair, :], trp[:, :QB, :])

                pend3 = []
                evacq = []
                streamB = QkStream(qkpsB)

                def queue_evac(done, lag):
                    # delay each pair's evac ~2 head-groups so its DVE
                    # mul chain completes before the PE transposes queue
                    if done is not None and done % 2 == 1:
                        evacq.append((done // G) * 2 + (done % G) // 2)
                    while len(evacq) > lag:
                        emit_evac(evacq.pop(0))

                for hg, done in emit_round(CH - 1, streamB, pvpsB, pend3):
                    queue_evac(done, 1)
                for done in flush_pend(pend3, pvpsB):
                    queue_evac(done, 1)
                queue_evac(None, 0)
                r3.close()

            if dbg:
                nc.sync.dma_start(dbg_t["dQT"][:], QT[:])
                nc.sync.dma_start(dbg_t["dKT"][:], KT[:])
                nc.sync.dma_start(dbg_t["dVst"][:], Vst[:])
                nc.sync.dma_start(dbg_t["dOacc"][:], Oacc[:])
                if upto >= 5:
                    nc.sync.dma_start(dbg_t["dOT"][:], OT[:])

            # ---- output projection + bias ----
            if upto >= 6:
                bpb = main.enter_context(
                    tc.tile_pool(name="bpbp", bufs=1)).tile(
                        [P, D], F32, tag="bpb")
                nc.sync.dma_start(bpb[:], bp[None, :].to_broadcast((P, D)))
                wpp = main.enter_context(tc.tile_pool(name="wpp", bufs=4))
                ops = main.enter_context(
                    tc.tile_pool(name="ops", bufs=3, space="PSUM"))
                osbp = main.enter_context(tc.tile_pool(name="osbp", bufs=3))
                for ob in range(4):
                    wts = []
                    for hf in range(2):
                        wt = wpp.tile([P, 8, NQ], BF16, tag="wph")
                        eng = nc.sync if hf == 0 else nc.scalar
                        eng.dma_start(wt[:],
                                      wp[:, hf * 8:(hf + 1) * 8, ob, :])
                        wts.append(wt)
                    for qb in range(QB):
                        op = ops.tile([P, NQ], F32, tag="op")
                        for jb in range(DB):
                            nc.tensor.matmul(
                                op[:], OT[:, jb, qb * P:(qb + 1) * P],
                                wts[jb // 8][:, jb % 8, :],
                                start=(jb == 0), stop=(jb == DB - 1))
                        osb = osbp.tile([P, NQ], BF16, tag="osb")
                        nc.vector.tensor_add(osb[:], op[:],
                                             bpb[:, ob * NQ:(ob + 1) * NQ])
                        nc.sync.dma_start(out[qb, :, ob, :], osb[:])

    nc.compile()
    return nc


_nc_cache = None


def _prep_inputs(x, Wq, Wkv, Wp, bp):
    """Host-side layout prep (bf16 casts, transposes, reshapes)."""
    import ml_dtypes
    bf16 = ml_dtypes.bfloat16
    x = np.asarray(x, dtype=np.float32)
    # Wq columns to g-major head order: j' = g*512 + h*64 + c, then to
    # partition-major [jc, p, db, j] so each jc-chunk is 1-2 big DMAs.
    Wq = (np.asarray(Wq, dtype=np.float32)
          .reshape(D, H, G, C).transpose(0, 2, 1, 3).reshape(D, D))
    wq_p = np.ascontiguousarray(
        Wq.reshape(DB, P, 4, NQ).transpose(2, 1, 0, 3)).astype(bf16)
    wkv_p = np.ascontiguousarray(
        np.asarray(Wkv, dtype=np.float32)
        .reshape(DB, P, 2, NQ).transpose(1, 0, 2, 3)).astype(bf16)
    wp_p = np.ascontiguousarray(
        np.asarray(Wp, dtype=np.float32)
        .reshape(DB, P, 4, NQ).transpose(1, 0, 2, 3)).astype(bf16)
    bp_p = np.ascontiguousarray(np.asarray(bp, dtype=np.float32))
    # x^T per batch: [d, n] -> partition-major [P, DB, N]
    xts = [np.ascontiguousarray(
               x[b].T.reshape(DB, P, N).transpose(1, 0, 2)).astype(bf16)
           for b in range(B)]
    return xts, wq_p, wkv_p, wp_p, bp_p


def make_in_maps(x, Wq, Wkv, Wp, bp):
    xts, wq_p, wkv_p, wp_p, bp_p = _prep_inputs(x, Wq, Wkv, Wp, bp)
    in_maps = []
    for c in range(8):
        b, qc = c // 4, c % 4
        # rotate the sequence axis so this core's query chunk is at n=0;
        # attention is invariant to a consistent permutation of the k/v axis
        xt_c = np.ascontiguousarray(np.roll(xts[b], -qc * NQ, axis=2))
        in_maps.append({
            "xt": xt_c,
            "wq": wq_p, "wkv": wkv_p, "wp": wp_p, "bp": bp_p,
        })
    return in_maps


def kernel(x, Wq, Wkv, Wp, bp):
    global _nc_cache
    if _nc_cache is None:
        _nc_cache = build_program()
    nc = _nc_cache
    in_maps = make_in_maps(x, Wq, Wkv, Wp, bp)
    res = run_bass_kernel_spmd(nc, in_maps, list(range(8)))
    outp = np.empty((B, N, D), np.float32)
    for c in range(8):
        b, qc = c // 4, c % 4
        o = np.asarray(res.results[c]["out"], dtype=np.float32)
        outp[b, qc * NQ:(qc + 1) * NQ] = o.transpose(0, 1, 2, 3).reshape(
            QB, P, D).reshape(NQ, D)
    return outp
